# revision 1
# baseline (speedup 1.0000x reference)
"""Trainium2 Bass kernel for nn_CNNNer (sparse band biaffine NER scorer).

Math collapse used here (everything after the GELU stage is linear):
  head = gelu(state@Wh+bh) ++ [1]          (features i = 0..200, i=200 is the 1)
  tail = gelu(state@Wt+bt) ++ [1]
  band[n,r,k] = head[n]^T U''_k tail[m],  m = n+r-64
      with U''_k = U_k + e_200 Wtp[k,:] + Whp[k,:]^T e_200^T
      (folds the h2/t2 additive terms of scores2 through the ones feature)
  scores'[n,r,t] = sum_k Wd[k,t] band_masked[n,r,k]
      masking zeroes whole head/tail feature columns (query/key validity),
      which commutes with the k-contraction, so
  scores'[n,r,t] = head_masked[n]^T UW_t tail_masked[m],
      UW_t = sum_k Wd[k,t] U''_k            (precomputed on host, [9,201,201])
  scores = scores' + bd  (host), masked-out entries = bd exactly.

Device work per core (8 cores; core = (batch b, query quarter) of 256 queries):
  1. headT/tailT = gelu MLPs computed transposed ([feature, position]).
  2. step A: UhT_t[j, x] = sum_i UW[t,i,j] headT[i,x]        (9 tags)
  3. step B: S_t[x, m]  = sum_j UhT_t[j, x] tailT[j, m]      (full 128x256
     score windows per query-chunk; band diag extracted on host)
"""

import os

import numpy as np

B, N, HID = 2, 1024, 768
BSZ = 200
W = 64
TAGS = 9
F = BSZ + 1  # 201 features incl the ones column
NQ = 256  # queries per core
NW = NQ + 2 * W  # 384 window positions per core
R = 2 * W + 1  # 129 band offsets
NCORES = 8
I2 = F - 128  # 73: second feature tile rows (i = 128..200)
F2 = BSZ - 128  # 72: second MLP output tile cols

_cache: dict = {}


def io_dt_name():
    return os.environ.get("BASSK_IO_DT", "f32r")


def _build_nc():
    import concourse.bass as bass
    import concourse.mybir as mybir
    import concourse.tile as tile
    from concourse import bacc

    dt = mybir.dt
    f32 = dt.float32
    io = {"f32": f32, "f32r": dt.float32r, "bf16": dt.bfloat16}[io_dt_name()]

    nc = bacc.Bacc(
        "TRN2", target_bir_lowering=False, debug=False, enable_asserts=False
    )
    xT = nc.dram_tensor("xT", [HID, NW], io, kind="ExternalInput").ap()
    wh = nc.dram_tensor("wh", [HID, BSZ], io, kind="ExternalInput").ap()
    wt = nc.dram_tensor("wt", [HID, BSZ], io, kind="ExternalInput").ap()
    # bias4 cols: bh[0:128], bt[0:128], bh[128:200]+pad, bt[128:200]+pad
    bias4 = nc.dram_tensor("bias4", [128, 4], f32, kind="ExternalInput").ap()
    # UW pre-arranged on host as [i, t, j] and split at i=128 so the loads
    # are plain row copies. j padded 201->204 so per-tag runs cannot merge
    # into descriptors over 1536B (those pin to a single DMA engine).
    FP = F + 3
    uw1d = nc.dram_tensor("uw1d", [128, TAGS, FP], io, kind="ExternalInput").ap()
    uw2d = nc.dram_tensor("uw2d", [I2, TAGS, FP], io, kind="ExternalInput").ap()
    # mask pre-broadcast on host: a partition-broadcast DMA lowers to
    # per-element descriptors and clogs the queue for ~25us
    msk = nc.dram_tensor("msk", [128, NW], io, kind="ExternalInput").ap()
    sout = nc.dram_tensor("sout", [TAGS, NQ, NQ], f32, kind="ExternalOutput").ap()

    gelu = {
        "gelu": mybir.ActivationFunctionType.Gelu,
        "identity": mybir.ActivationFunctionType.Identity,
    }[os.environ.get("BASSK_ACT", "gelu")]

    with tile.TileContext(nc) as tc:
        with (
            tc.tile_pool(name="sb", bufs=1) as sb,
            tc.tile_pool(name="ps_mlp", bufs=2, space="PSUM") as ps_mlp,
            tc.tile_pool(name="ps_a", bufs=2, space="PSUM") as ps_a,
            tc.tile_pool(name="ps_s", bufs=4, space="PSUM") as ps_s,
        ):
            # ---- loads (spread across DGE queues; x/weights split so the
            # MLP matmuls can start on the first chunks; uw queued behind
            # them so its transfer overlaps MLP compute) ----
            # One dma_start's descriptor chain runs on a single DMA engine
            # (~22.5 GB/s), so split each sizable load into pieces that run
            # on separate engines concurrently.
            qs = (nc.sync, nc.scalar)
            xTr = xT.rearrange("(ht p) c -> p ht c", p=128)
            x_sb = sb.tile([128, 6, NW], io)
            nc.sync.dma_start(out=x_sb[:, 0:3, :], in_=xTr[:, 0:3, :])
            nc.scalar.dma_start(out=x_sb[:, 3:6, :], in_=xTr[:, 3:6, :])
            whr = wh.rearrange("(ht p) m -> p ht m", p=128)
            wtr = wt.rearrange("(ht p) m -> p ht m", p=128)
            wh_sb = sb.tile([128, 6, BSZ], io)
            wt_sb = sb.tile([128, 6, BSZ], io)
            nc.sync.dma_start(out=wh_sb, in_=whr)
            nc.scalar.dma_start(out=wt_sb, in_=wtr)
            m_sb = sb.tile([128, NW], io)
            nc.gpsimd.dma_start(out=m_sb, in_=msk)
            b_sb = sb.tile([128, 4], f32)
            nc.gpsimd.dma_start(out=b_sb, in_=bias4)
            uw1 = sb.tile([128, TAGS, F], io)
            uw2 = sb.tile([I2, TAGS, F], io)
            nc.sync.dma_start(out=uw1, in_=uw1d[:, :, 0:F])
            nc.scalar.dma_start(out=uw2, in_=uw2d[:, :, 0:F])
            bh1, bt1 = b_sb[:, 0:1], b_sb[:, 1:2]
            bh2, bt2 = b_sb[0:F2, 2:3], b_sb[0:F2, 3:4]

            headT1 = sb.tile([128, NQ], io)
            headT2 = sb.tile([I2, NQ], io)
            tailT1 = sb.tile([128, NW], io)
            tailT2 = sb.tile([I2, NW], io)
            uh1 = sb.tile([128, TAGS, NQ], io)
            uh2 = sb.tile([I2, TAGS, NQ], io)
            s_sb0 = sb.tile([128, TAGS, NQ], f32)
            s_sb1 = sb.tile([128, TAGS, NQ], f32)

            # ---- MLPs: o = gelu(W^T x + b), computed transposed ----
            for w_sb, b1, b2, o1, o2, c0, ncols in (
                (wh_sb, bh1, bh2, headT1, headT2, W, NQ),
                (wt_sb, bt1, bt2, tailT1, tailT2, 0, NW),
            ):
                for fw, f0, o, bias in ((128, 0, o1, b1), (F2, 128, o2, b2)):
                    pm = ps_mlp.tile([fw, ncols], f32, tag="pm")
                    for ht in range(6):
                        nc.tensor.matmul(
                            pm,
                            w_sb[:, ht, f0 : f0 + fw],
                            x_sb[:, ht, c0 : c0 + ncols],
                            start=(ht == 0),
                            stop=(ht == 5),
                        )
                    nc.scalar.activation(out=o[0:fw, :], in_=pm, func=gelu, bias=bias)
                # mask all columns; ones feature row (i == 200) is the mask
                # row itself, DMA'd in (engines can't address partition 72)
                nc.vector.tensor_mul(o1, o1, m_sb[0:128, c0 : c0 + ncols])
                nc.vector.tensor_mul(
                    o2[0:F2, :], o2[0:F2, :], m_sb[0:F2, c0 : c0 + ncols]
                )
                nc.gpsimd.dma_start(
                    out=o2[F2 : F2 + 1, :], in_=msk[0:1, c0 : c0 + ncols]
                )

            # ---- step A: UhT_t[j, x] = sum_i UW[t,i,j] headT[i,x] ----
            for t in range(TAGS):
                for jw, j0, uh in ((128, 0, uh1), (I2, 128, uh2)):
                    pa = ps_a.tile([jw, NQ], f32, tag="pa")
                    for it, (u_sb, h_sb) in enumerate(
                        ((uw1, headT1), (uw2, headT2))
                    ):
                        nc.tensor.matmul(
                            pa,
                            u_sb[:, t, j0 : j0 + jw],
                            h_sb,
                            start=(it == 0),
                            stop=(it == 1),
                        )
                    nc.any.tensor_copy(uh[:, t, :], pa)

            # ---- step B: S_t[x, m] = sum_j UhT_t[j, x] tailT[j, m] ----
            for qc in range(2):
                s_sb = (s_sb0, s_sb1)[qc]
                for t in range(TAGS):
                    pS = ps_s.tile([128, NQ], f32, tag="ps")
                    for jt, (uh, tl) in enumerate(((uh1, tailT1), (uh2, tailT2))):
                        nc.tensor.matmul(
                            pS,
                            uh[:, t, qc * 128 : qc * 128 + 128],
                            tl[:, qc * 128 : qc * 128 + NQ],
                            start=(jt == 0),
                            stop=(jt == 1),
                        )
                    nc.any.tensor_copy(s_sb[:, t, :], pS)
                    if t % 3 == 2:
                        # store finished tag-triples so writeback overlaps
                        # the remaining compute
                        qs[(qc + t) % 2].dma_start(
                            out=sout[
                                t - 2 : t + 1, qc * 128 : (qc + 1) * 128, :
                            ].transpose([1, 0, 2]),
                            in_=s_sb[:, t - 2 : t + 1, :],
                        )

    nc.compile()
    return nc


def _np_io_dt():
    if io_dt_name() == "bf16":
        import ml_dtypes

        return ml_dtypes.bfloat16
    return np.float32


def _get_nc():
    key = "nc-" + io_dt_name()
    if key not in _cache:
        _cache[key] = _build_nc()
    return _cache[key]


def _install_ntff_hook():
    """Profiling-only (BASSK_TRACE=1): provide antenv.axon_hooks if the
    image lacks it, wired to the libaxon NTFF capture via ctypes."""
    import sys
    import types

    try:
        from antenv.axon_hooks import get_axon_ntff_profile_hook  # noqa: F401

        return
    except ImportError:
        pass
    from trn_agent_boot.trn_boot import _ntff_profile_via_ctypes

    hook = _ntff_profile_via_ctypes("/opt/axon/libaxon_pjrt.so")
    mod = types.ModuleType("antenv.axon_hooks")
    mod._hook = hook
    mod.get_axon_ntff_profile_hook = lambda: mod._hook
    mod.set_axon_ntff_profile_hook = lambda h: setattr(mod, "_hook", h)
    sys.modules["antenv.axon_hooks"] = mod


def _host_prep(state, lengths, Wh, bh, Wt, bt, U, Wcat, Wd):
    """Fold U/Wcat/Wd into UW[9,201,201] and build per-core inputs."""
    Whp = Wcat[:, :F]  # [K, 201]
    Wtp = Wcat[:, F:]  # [K, 201]
    U2 = U.astype(np.float64).copy()
    U2[:, F - 1, :] += Wtp  # head ones-row picks up the tail term
    U2[:, :, F - 1] += Whp  # tail ones-col picks up the head term
    UW = np.einsum("kt,kij->tij", Wd.astype(np.float64), U2).astype(np.float32)
    UW = np.ascontiguousarray(UW)

    in_maps = []
    for b in range(B):
        for qi in range(N // NQ):
            q0 = qi * NQ
            lo = q0 - W
            xw = np.zeros((NW, HID), np.float32)
            s, e = max(lo, 0), min(q0 + NQ + W, N)
            xw[s - lo : e - lo] = state[b, s:e]
            pos = lo + np.arange(NW)
            mrow = ((pos >= 0) & (pos < N) & (pos < lengths[b])).astype(np.float32)
            iodt = _np_io_dt()
            uwp = np.zeros((F, TAGS, F + 3), UW.dtype)
            uwp[:, :, 0:F] = UW.transpose(1, 0, 2)
            uwr = uwp.astype(iodt)
            in_maps.append(
                {
                    "xT": np.ascontiguousarray(xw.T).astype(iodt),
                    "wh": Wh.astype(iodt),
                    "wt": Wt.astype(iodt),
                    "bias4": np.ascontiguousarray(
                        np.stack(
                            [
                                bh[0:128],
                                bt[0:128],
                                np.pad(bh[128:BSZ], (0, 128 - F2)),
                                np.pad(bt[128:BSZ], (0, 128 - F2)),
                            ],
                            axis=1,
                        ).astype(np.float32)
                    ),
                    "uw1d": np.ascontiguousarray(uwr[0:128]),
                    "uw2d": np.ascontiguousarray(uwr[128:F]),
                    "msk": np.ascontiguousarray(
                        np.broadcast_to(mrow[None, :], (128, NW))
                    ).astype(iodt),
                }
            )
    return in_maps


def _assemble(outs, bd):
    """outs: NCORES arrays [TAGS, NQ, NQ] -> scores [B, N, R, TAGS]."""
    scores = np.empty((B, N, R, TAGS), np.float32)
    mi = (np.arange(NQ) % 128)[:, None] + np.arange(R)[None, :]
    for c, S in enumerate(outs):
        b, qi = divmod(c, N // NQ)
        g = np.take_along_axis(S, mi[None, :, :], axis=2)
        scores[b, qi * NQ : (qi + 1) * NQ] = g.transpose(1, 2, 0)
    scores += bd.astype(np.float32)[None, None, None, :]
    return np.where(np.isfinite(scores), scores, 0.0).astype(np.float32)


def kernel(**inputs):
    state = np.asarray(inputs["state"], np.float32)
    lengths = np.asarray(inputs["lengths"]).astype(np.int64)
    Wh = np.ascontiguousarray(np.asarray(inputs["Wh"], np.float32))
    bh = np.asarray(inputs["bh"], np.float32)
    Wt = np.ascontiguousarray(np.asarray(inputs["Wt"], np.float32))
    bt = np.asarray(inputs["bt"], np.float32)
    U = np.asarray(inputs["U"], np.float32)
    Wcat = np.asarray(inputs["Wcat"], np.float32)
    Wd = np.asarray(inputs["Wd"], np.float32)
    bd = np.asarray(inputs["bd"], np.float32)

    in_maps = _host_prep(state, lengths, Wh, bh, Wt, bt, U, Wcat, Wd)
    nc = _get_nc()

    if os.environ.get("BASSK_SIM"):
        from concourse.bass_interp import CoreSim

        outs = []
        for im in in_maps:
            sim = CoreSim(nc, trace=False)
            for k, v in im.items():
                sim.tensor(k)[:] = v
            sim.simulate()
            outs.append(sim.tensor("sout").copy())
    else:
        trace = bool(os.environ.get("BASSK_TRACE"))
        if trace:
            _install_ntff_hook()
        from concourse.bass_utils import run_bass_kernel_spmd

        try:
            res = run_bass_kernel_spmd(
                nc, in_maps, core_ids=list(range(NCORES)), trace=trace
            )
        except Exception:
            # transient NRT/device hiccups recover on a fresh attempt
            import time

            time.sleep(2.0)
            res = run_bass_kernel_spmd(
                nc, in_maps, core_ids=list(range(NCORES)), trace=trace
            )
        _cache["last_result"] = res
        outs = [r["sout"] for r in res.results]

    return _assemble(outs, bd)



# revision 5
# speedup vs baseline: 1.2416x; 1.2416x over previous
"""Trainium2 Bass kernel for nn_CNNNer (sparse band biaffine NER scorer).

Math collapse (everything after the GELU stage is linear):
  head = gelu(state@Wh+bh) ++ [1]          (features 0..200, 200 is the 1)
  tail = gelu(state@Wt+bt) ++ [1]
  band[n,r,k] = head[n]^T U''_k tail[m],  m = n+r-64
      with U''_k = U_k + e_200 Wtp[k,:] + Whp[k,:]^T e_200^T
  scores'[n,r,t] = head[n]^T UW_t tail[m],  UW_t = sum_k Wd[k,t] U''_k
      (UW precomputed on host, [9,201,201]); scores = scores' + bd.
  Query/key padding masks zero whole band entries independently, so ALL
  masking is applied on host after the gather (masked entries := bd).

Device work per core (8 cores; core = (batch b, query quarter), 256 queries):
  1. headT/tailT = gelu MLPs, transposed ([feature, position]), bf16.
  2. step A: UhT_t[j, x] = sum_i UW[t,i,j] headT[i,x]        (9 tags)
  3. step B: S_t[x, m]  = sum_j UhT_t[j, x] tailT[j, m]      (two 128x256
     windows per core; band diagonals extracted on host)

Feature dim F=201 is chunked (104, 97); the constant-1 feature sits at
local partition 96 of chunk 2 (96 is engine-alignment-legal for memset).

DMA design (v2): every bulk transfer is a 128-partition DMA whose
contiguous runs stay <= 1536B (runs beyond that pin the whole chain to a
single DMA engine; <128-partition transfers also pin). UW is zero-padded
to 128 partitions and group-packed so descriptors are ~1224B. Loads are
split across the sync/scalar/vector/gpsimd queues in critical-path order
(x+w first, uw behind), output is stored bf16 per 3 tags (1536B rows).
"""

import os

import numpy as np

B, N, HID = 2, 1024, 768
BSZ = 200
W = 64
TAGS = 9
F = BSZ + 1  # 201 features incl the ones column
NQ = 256  # queries per core
NW = NQ + 2 * W  # 384 window positions per core
R = 2 * W + 1  # 129 band offsets
NCORES = 8
C1 = 104  # feature chunk 1: i/j = 0..103
C2 = 97  # feature chunk 2: i/j = 104..200 (local 96 = ones row)
C2G = 96  # gelu rows of chunk 2 (features 104..199)
JP = 204  # padded j-row length inside uw groups
HT = 6  # 768/128 contraction chunks

_cache: dict = {}


def io_dt_name():
    return os.environ.get("BASSK_IO_DT", "bf16")


def _lay(io_name):
    """DMA group layout params: keep contiguous runs <= 1536B."""
    es = 2 if io_name == "bf16" else 4
    xg = 1536 // (NW * es)  # ht rows per x group (2 bf16 / 1 f32)
    wg = 1536 // es // 200 * 200  # w elems per group (600 / 300)
    ur = 1536 // (JP * es)  # uw rows per group (3 bf16 / 1 f32)
    ts = 1536 // (NQ * es)  # tags per output store (3 / 1)
    assert 600 % wg == 0 or wg == 600
    return es, xg, wg, ur, ts


def _build_nc():
    import concourse.mybir as mybir
    import concourse.tile as tile
    from concourse import bacc

    dt = mybir.dt
    f32 = dt.float32
    ion = io_dt_name()
    io = {"f32": f32, "f32r": dt.float32r, "bf16": dt.bfloat16}[ion]
    es, XG, WG, UR, TS = _lay(ion)
    XNG, WNG, UNG = HT // XG, 2400 // WG, 18 // UR

    nc = bacc.Bacc(
        "TRN2", target_bir_lowering=False, debug=False, enable_asserts=False
    )
    xd = nc.dram_tensor("xd", [128, XNG, XG * NW + 8], io, kind="ExternalInput").ap()
    wd = nc.dram_tensor("wd", [128, WNG, WG + 8], io, kind="ExternalInput").ap()
    uwd = nc.dram_tensor("uwd", [128, UNG, UR * JP + 16], io, kind="ExternalInput").ap()
    # bias cols: bh[0:104], bt[0:104], bh[104:200]+pad, bt[104:200]+pad
    bias4 = nc.dram_tensor("bias4", [128, 4], f32, kind="ExternalInput").ap()
    sout = nc.dram_tensor("sout", [2, 128, TAGS, NQ], io, kind="ExternalOutput").ap()

    gelu = {
        "gelu": mybir.ActivationFunctionType.Gelu,
        "identity": mybir.ActivationFunctionType.Identity,
    }[os.environ.get("BASSK_ACT", "gelu")]

    with tile.TileContext(nc) as tc:
        with (
            tc.tile_pool(name="sb", bufs=1) as sb,
            tc.tile_pool(name="ps_mlp", bufs=2, space="PSUM") as ps_mlp,
            tc.tile_pool(name="ps_a", bufs=2, space="PSUM") as ps_a,
            tc.tile_pool(name="ps_s", bufs=2, space="PSUM") as ps_s,
        ):
            # ---- SBUF tiles (flat free dims; slicing helpers below) ----
            x_sb = sb.tile([128, XNG, XG * NW], io)
            w_sb = sb.tile([128, WNG, WG], io)
            uw_sb = sb.tile([128, UNG, UR * JP], io)
            b_sb = sb.tile([128, 4], f32)

            def xsl(ht, c0, ncols):
                g, e = divmod(ht * NW + c0, XG * NW)
                assert e + ncols <= XG * NW
                return x_sb[:, g, e : e + ncols]

            def wsl(off, n):
                g, e = divmod(off, WG)
                assert e + n <= WG
                return w_sb[:, g, e : e + n]

            def usl(t, c, j0, jw):
                g, k = divmod(2 * t + c, UR)
                part = C1 if c == 0 else C2
                return uw_sb[0:part, g, k * JP + j0 : k * JP + j0 + jw]

            # ---- loads, split across queues in critical-path order ----
            # sync: x (per-group for fine waits), then uw tags 0-2
            for g in range(XNG):
                nc.sync.dma_start(
                    out=x_sb[:, g : g + 1, :], in_=xd[:, g : g + 1, 0 : XG * NW]
                )
            # scalar: w first (gates MLP), then uw tags 3-5
            wsplit = (WNG // 3) or 1
            nc.scalar.dma_start(
                out=w_sb[:, 0:wsplit, :], in_=wd[:, 0:wsplit, 0:WG]
            )
            nc.scalar.dma_start(
                out=w_sb[:, wsplit:WNG, :], in_=wd[:, wsplit:WNG, 0:WG]
            )
            u3 = UNG // 3
            nc.gpsimd.dma_start(out=b_sb, in_=bias4)
            nc.gpsimd.dma_start(
                out=uw_sb[:, 2 * u3 : UNG, :], in_=uwd[:, 2 * u3 : UNG, 0 : UR * JP]
            )
            nc.sync.dma_start(out=uw_sb[:, 0:u3, :], in_=uwd[:, 0:u3, 0 : UR * JP])
            nc.scalar.dma_start(
                out=uw_sb[:, u3 : 2 * u3, :], in_=uwd[:, u3 : 2 * u3, 0 : UR * JP]
            )

            headT1 = sb.tile([C1, NQ], io)
            headT2 = sb.tile([C2, NQ], io)
            tailT1 = sb.tile([C1, NW], io)
            tailT2 = sb.tile([C2, NW], io)
            uh1 = sb.tile([C1, TAGS, NQ], io)
            uh2 = sb.tile([C2, TAGS, NQ], io)
            s_sb0 = sb.tile([128, TAGS, NQ], io)
            s_sb1 = sb.tile([128, TAGS, NQ], io)

            # ---- MLPs: o = gelu(W^T x + b), transposed; ones via memset ----
            for woff, c0, ncols, o1, o2, bc in (
                (0, W, NQ, headT1, headT2, 0),
                (200, 0, NW, tailT1, tailT2, 1),
            ):
                pm1 = ps_mlp.tile([C1, ncols], f32, tag="pm1")
                pm2 = ps_mlp.tile([C2G, ncols], f32, tag="pm2")
                for ht in range(HT):
                    xa = xsl(ht, c0, ncols)
                    nc.tensor.matmul(
                        pm1, wsl(ht * 400 + woff, C1), xa,
                        start=(ht == 0), stop=(ht == HT - 1),
                    )
                    nc.tensor.matmul(
                        pm2, wsl(ht * 400 + woff + C1, C2G), xa,
                        start=(ht == 0), stop=(ht == HT - 1),
                    )
                nc.scalar.activation(
                    out=o1[0:C1, :], in_=pm1, func=gelu, bias=b_sb[0:C1, bc : bc + 1]
                )
                nc.scalar.activation(
                    out=o2[0:C2G, :], in_=pm2, func=gelu,
                    bias=b_sb[0:C2G, bc + 2 : bc + 3],
                )
                nc.vector.memset(o2[C2G:C2, :], 1.0)

            # ---- step A: UhT_t[j, x] = sum_i UW[t,i,j] headT[i,x] ----
            for t in range(TAGS):
                for j0, jw, uh in ((0, C1, uh1), (C1, C2, uh2)):
                    pa = ps_a.tile([jw, NQ], f32, tag="pa")
                    nc.tensor.matmul(
                        pa, usl(t, 0, j0, jw), headT1, start=True, stop=False
                    )
                    nc.tensor.matmul(
                        pa, usl(t, 1, j0, jw), headT2, start=False, stop=True
                    )
                    nc.any.tensor_copy(uh[:, t, :], pa)

            # ---- step B: S_t[x, m] = sum_j UhT_t[j, x] tailT[j, m] ----
            qs = (nc.sync, nc.scalar)
            for qc in range(2):
                s_sb = (s_sb0, s_sb1)[qc]
                for t in range(TAGS):
                    pS = ps_s.tile([128, NQ], f32, tag="ps")
                    nc.tensor.matmul(
                        pS, uh1[:, t, qc * 128 : qc * 128 + 128],
                        tailT1[:, qc * 128 : qc * 128 + NQ],
                        start=True, stop=False,
                    )
                    nc.tensor.matmul(
                        pS, uh2[:, t, qc * 128 : qc * 128 + 128],
                        tailT2[:, qc * 128 : qc * 128 + NQ],
                        start=False, stop=True,
                    )
                    nc.any.tensor_copy(s_sb[:, t, :], pS)
                    if t % TS == TS - 1:
                        qs[(qc + t // TS) % 2].dma_start(
                            out=sout[qc, :, t - TS + 1 : t + 1, :],
                            in_=s_sb[:, t - TS + 1 : t + 1, :],
                        )

    nc.compile()
    return nc


def _np_io_dt():
    if io_dt_name() == "bf16":
        import ml_dtypes

        return ml_dtypes.bfloat16
    return np.float32


def _get_nc():
    key = "nc-" + io_dt_name()
    if key not in _cache:
        _cache[key] = _build_nc()
    return _cache[key]


def _install_ntff_hook():
    """Profiling-only (BASSK_TRACE=1): provide antenv.axon_hooks if the
    image lacks it, wired to the libaxon NTFF capture via ctypes."""
    import sys
    import types

    try:
        from antenv.axon_hooks import get_axon_ntff_profile_hook  # noqa: F401

        return
    except ImportError:
        pass
    from trn_agent_boot.trn_boot import _ntff_profile_via_ctypes

    hook = _ntff_profile_via_ctypes("/opt/axon/libaxon_pjrt.so")
    mod = types.ModuleType("antenv.axon_hooks")
    mod._hook = hook
    mod.get_axon_ntff_profile_hook = lambda: mod._hook
    mod.set_axon_ntff_profile_hook = lambda h: setattr(mod, "_hook", h)
    sys.modules["antenv.axon_hooks"] = mod


def _host_prep(state, Wh, bh, Wt, bt, U, Wcat, Wd):
    """Fold U/Wcat/Wd into UW[9,201,201]; pack DMA-friendly per-core inputs."""
    iodt = _np_io_dt()
    es, XG, WG, UR, TS = _lay(io_dt_name())
    XNG, WNG, UNG = HT // XG, 2400 // WG, 18 // UR

    Whp = Wcat[:, :F]
    Wtp = Wcat[:, F:]
    U2 = U.astype(np.float64).copy()
    U2[:, F - 1, :] += Wtp  # head ones-row picks up the tail term
    U2[:, :, F - 1] += Whp  # tail ones-col picks up the head term
    UW = np.einsum("kt,kij->tij", Wd.astype(np.float64), U2).astype(np.float32)

    # uwd[p, g, k*JP + j] = UW[t, ioff_c + p, j], (t,c) = divmod(g*UR+k+..)
    uwd = np.zeros((128, UNG, UR * JP + 16), np.float32)
    for t in range(TAGS):
        for c, (ioff, part) in enumerate(((0, C1), (C1, C2))):
            g, k = divmod(2 * t + c, UR)
            uwd[0:part, g, k * JP : k * JP + F] = UW[t, ioff : ioff + part, :].T.T
    uwd = np.ascontiguousarray(uwd.astype(iodt))

    # wflat[p, ht*400 + {0,200}] = Wh/Wt[ht*128+p, :]
    wflat = np.zeros((128, 2400), np.float32)
    for ht in range(HT):
        wflat[:, ht * 400 : ht * 400 + 200] = Wh[ht * 128 : (ht + 1) * 128, :]
        wflat[:, ht * 400 + 200 : ht * 400 + 400] = Wt[ht * 128 : (ht + 1) * 128, :]
    wdense = np.zeros((128, WNG, WG + 8), np.float32)
    wdense[:, :, 0:WG] = wflat.reshape(128, WNG, WG)
    wdense = np.ascontiguousarray(wdense.astype(iodt))

    bias4 = np.zeros((128, 4), np.float32)
    bias4[0:C1, 0] = bh[0:C1]
    bias4[0:C1, 1] = bt[0:C1]
    bias4[0:C2G, 2] = bh[C1:BSZ]
    bias4[0:C2G, 3] = bt[C1:BSZ]

    in_maps = []
    for b in range(B):
        for qi in range(N // NQ):
            q0 = qi * NQ
            lo = q0 - W
            xw = np.zeros((NW, HID), np.float32)
            s, e = max(lo, 0), min(q0 + NQ + W, N)
            xw[s - lo : e - lo] = state[b, s:e]
            # xflat[p, ht*NW + c] = xw[c, ht*128+p]
            xflat = (
                xw.T.reshape(HT, 128, NW).transpose(1, 0, 2).reshape(128, HT * NW)
            )
            xdg = np.zeros((128, XNG, XG * NW + 8), np.float32)
            xdg[:, :, 0 : XG * NW] = xflat.reshape(128, XNG, XG * NW)
            in_maps.append(
                {
                    "xd": np.ascontiguousarray(xdg.astype(iodt)),
                    "wd": wdense,
                    "uwd": uwd,
                    "bias4": bias4,
                }
            )
    return in_maps


def _assemble(outs, bd, lengths):
    """outs: NCORES arrays [2, 128, TAGS, NQ] -> scores [B, N, R, TAGS]."""
    scores = np.empty((B, N, R, TAGS), np.float32)
    mi = np.arange(128)[:, None] + np.arange(R)[None, :]  # [128, R]
    for c, S in enumerate(outs):
        b, qi = divmod(c, N // NQ)
        for qc in range(2):
            g = np.take_along_axis(
                S[qc].astype(np.float32), mi[:, None, :], axis=2
            )  # [128, TAGS, R]
            scores[b, qi * NQ + qc * 128 : qi * NQ + (qc + 1) * 128] = g.transpose(
                0, 2, 1
            )
    bdf = bd.astype(np.float32)
    scores += bdf[None, None, None, :]
    # host-side pad mask: masked entries equal bd exactly (0 @ Wd + bd)
    j_idx = np.arange(N)[:, None] + np.arange(R)[None, :] - W  # [N, R]
    in_range = (j_idx >= 0) & (j_idx < N)
    for b in range(B):
        key_ok = in_range & (j_idx < lengths[b])
        q_ok = np.arange(N) < lengths[b]
        pad = ~(key_ok & q_ok[:, None])  # [N, R]
        scores[b][pad] = bdf
    return np.where(np.isfinite(scores), scores, 0.0).astype(np.float32)


def kernel(**inputs):
    state = np.asarray(inputs["state"], np.float32)
    lengths = np.asarray(inputs["lengths"]).astype(np.int64)
    Wh = np.ascontiguousarray(np.asarray(inputs["Wh"], np.float32))
    bh = np.asarray(inputs["bh"], np.float32)
    Wt = np.ascontiguousarray(np.asarray(inputs["Wt"], np.float32))
    bt = np.asarray(inputs["bt"], np.float32)
    U = np.asarray(inputs["U"], np.float32)
    Wcat = np.asarray(inputs["Wcat"], np.float32)
    Wd = np.asarray(inputs["Wd"], np.float32)
    bd = np.asarray(inputs["bd"], np.float32)

    in_maps = _host_prep(state, Wh, bh, Wt, bt, U, Wcat, Wd)
    nc = _get_nc()

    if os.environ.get("BASSK_SIM"):
        from concourse.bass_interp import CoreSim

        outs = []
        for im in in_maps[: int(os.environ.get("BASSK_SIM_N", len(in_maps)))]:
            sim = CoreSim(nc, trace=False)
            for k, v in im.items():
                sim.tensor(k)[:] = v
            sim.simulate()
            outs.append(sim.tensor("sout").copy())
        while len(outs) < NCORES:
            outs.append(outs[-1])
    else:
        trace = bool(os.environ.get("BASSK_TRACE"))
        if trace:
            _install_ntff_hook()
        from concourse.bass_utils import run_bass_kernel_spmd

        try:
            res = run_bass_kernel_spmd(
                nc, in_maps, core_ids=list(range(NCORES)), trace=trace
            )
        except Exception:
            # transient NRT/device hiccups recover on a fresh attempt
            import time

            time.sleep(2.0)
            res = run_bass_kernel_spmd(
                nc, in_maps, core_ids=list(range(NCORES)), trace=trace
            )
        _cache["last_result"] = res
        outs = [r["sout"] for r in res.results]

    return _assemble(outs, bd, lengths)


# revision 7
# speedup vs baseline: 1.5159x; 1.2209x over previous
"""Trainium2 Bass kernel for nn_CNNNer (sparse band biaffine NER scorer).

Math collapse (everything after the GELU stage is linear):
  head = gelu(state@Wh+bh) ++ [1]          (features 0..200, 200 is the 1)
  tail = gelu(state@Wt+bt) ++ [1]
  band[n,r,k] = head[n]^T U''_k tail[m],  m = n+r-64
      with U''_k = U_k + e_200 Wtp[k,:] + Whp[k,:]^T e_200^T
  scores'[n,r,t] = head[n]^T UW_t tail[m],  UW_t = sum_k Wd[k,t] U''_k
      (UW precomputed on host, [9,201,201]); scores = scores' + bd.
  Query/key padding masks zero whole band entries independently, so ALL
  masking is applied on host after the gather (masked entries := bd).

Device work per core (8 cores; core = (batch b, query quarter), 256 queries):
  1. headT/tailT = gelu MLPs, transposed ([feature, position]), bf16.
  2. step A: UhT_t[j, x] = sum_i UW[t,i,j] headT[i,x]        (9 tags)
  3. step B: S_t[x, m]  = sum_j UhT_t[j, x] tailT[j, m]      (two 128x256
     windows per core; band diagonals extracted on host)

Feature dim F=201 is chunked (104, 97); the constant-1 feature sits at
local partition 96 of chunk 2 (96 is engine-alignment-legal for memset).

DMA design (v3): all transfers are 128-partition DMAs with FLAT per-
partition-contiguous layouts — descriptor-size probe showed bigger runs
are faster (400B/ns at 6KB vs 190B/ns at 1.5KB) and spread across all 16
engines; only <128-partition DMAs pin to one engine. UW is zero-padded
to 128 partitions. Loads split across sync/scalar (+gpsimd for the last
uw third) in critical-path order. PSUM pools are scoped per phase, and
dummy warm-up matmuls ramp the PE p-state during the initial DMA wait.
"""

import os

import numpy as np

B, N, HID = 2, 1024, 768
BSZ = 200
W = 64
TAGS = 9
F = BSZ + 1  # 201 features incl the ones column
NQ = 256  # queries per core
NW = NQ + 2 * W  # 384 window positions per core
R = 2 * W + 1  # 129 band offsets
NCORES = 8
C1 = 104  # feature chunk 1: i/j = 0..103
C2 = 97  # feature chunk 2: i/j = 104..200 (local 96 = ones row)
C2G = 96  # gelu rows of chunk 2 (features 104..199)
JP = 204  # j-row stride inside uw rows
HT = 6  # 768/128 contraction chunks
TS = 3  # tags per output store
NWARM = 14  # PE p-state warm-up matmuls

_cache: dict = {}


def io_dt_name():
    return os.environ.get("BASSK_IO_DT", "bf16")


def _build_nc():
    import concourse.mybir as mybir
    import concourse.tile as tile
    from concourse import bacc

    dt = mybir.dt
    f32 = dt.float32
    ion = io_dt_name()
    io = {"f32": f32, "f32r": dt.float32r, "bf16": dt.bfloat16}[ion]

    nc = bacc.Bacc(
        "TRN2", target_bir_lowering=False, debug=False, enable_asserts=False
    )
    # flat per-partition layouts (see module docstring)
    xd = nc.dram_tensor("xd", [128, HT * NW], io, kind="ExternalInput").ap()
    wd = nc.dram_tensor("wd", [128, 2400], io, kind="ExternalInput").ap()
    uwd = nc.dram_tensor("uwd", [128, 18 * JP], io, kind="ExternalInput").ap()
    # bias cols: bh[0:104], bt[0:104], bh[104:200]+pad, bt[104:200]+pad
    bias4 = nc.dram_tensor("bias4", [128, 4], f32, kind="ExternalInput").ap()
    sout = nc.dram_tensor("sout", [2, 128, TAGS, NQ], io, kind="ExternalOutput").ap()

    gelu = {
        "gelu": mybir.ActivationFunctionType.Gelu,
        "identity": mybir.ActivationFunctionType.Identity,
    }[os.environ.get("BASSK_ACT", "gelu")]

    with tile.TileContext(nc) as tc:
        with tc.tile_pool(name="sb", bufs=1) as sb:
            x_sb = sb.tile([128, HT * NW], io)
            w_sb = sb.tile([128, 2400], io)
            uw_sb = sb.tile([128, 18 * JP], io)
            b_sb = sb.tile([128, 4], f32)
            scr = sb.tile([128, NQ], io)

            def xsl(ht, c0, ncols):
                off = ht * NW + c0
                return x_sb[:, off : off + ncols]

            def wsl(off, n):
                return w_sb[:, off : off + n]

            def usl(t, c, j0, jw):
                part = C1 if c == 0 else C2
                off = (2 * t + c) * JP + j0
                return uw_sb[0:part, off : off + jw]

            # ---- loads: critical-path order across queues ----
            # sync: x halves, then uw tags 0-2; scalar: w halves, then
            # uw tags 3-5; gpsimd (SWDGE, blocks the engine): bias + rest
            H3 = HT * NW // 2
            nc.sync.dma_start(out=x_sb[:, 0:H3], in_=xd[:, 0:H3])
            nc.scalar.dma_start(out=w_sb[:, 0:1200], in_=wd[:, 0:1200])
            nc.sync.dma_start(out=x_sb[:, H3 : 2 * H3], in_=xd[:, H3 : 2 * H3])
            nc.scalar.dma_start(out=w_sb[:, 1200:2400], in_=wd[:, 1200:2400])
            nc.gpsimd.dma_start(out=b_sb, in_=bias4)
            U3 = 6 * JP
            nc.gpsimd.dma_start(
                out=uw_sb[:, 2 * U3 : 3 * U3], in_=uwd[:, 2 * U3 : 3 * U3]
            )
            nc.sync.dma_start(out=uw_sb[:, 0:U3], in_=uwd[:, 0:U3])
            nc.scalar.dma_start(out=uw_sb[:, U3 : 2 * U3], in_=uwd[:, U3 : 2 * U3])

            headT1 = sb.tile([C1, NQ], io)
            headT2 = sb.tile([C2, NQ], io)
            tailT1 = sb.tile([C1, NW], io)
            tailT2 = sb.tile([C2, NW], io)
            uh1 = sb.tile([C1, TAGS, NQ], io)
            uh2 = sb.tile([C2, TAGS, NQ], io)
            s_sb0 = sb.tile([128, TAGS, NQ], io)
            s_sb1 = sb.tile([128, TAGS, NQ], io)

            # ---- PE p-state warm-up during the DMA wait ----
            nc.vector.memset(scr, 0.0)
            with tc.tile_pool(name="ps_w", bufs=1, space="PSUM") as ps_w:
                pw = ps_w.tile([128, NQ], f32, tag="pw")
                for i in range(NWARM):
                    nc.tensor.matmul(
                        pw, scr[:, 0:128], scr,
                        start=(i == 0), stop=(i == NWARM - 1),
                    )

            # ---- MLPs: o = gelu(W^T x + b), transposed; ones via memset ----
            with tc.tile_pool(name="ps_mlp", bufs=2, space="PSUM") as ps_mlp:
                for woff, c0, ncols, o1, o2, bc in (
                    (0, W, NQ, headT1, headT2, 0),
                    (200, 0, NW, tailT1, tailT2, 1),
                ):
                    pm1 = ps_mlp.tile([C1, ncols], f32, tag="pm1")
                    pm2 = ps_mlp.tile([C2G, ncols], f32, tag="pm2")
                    for ht in range(HT):
                        xa = xsl(ht, c0, ncols)
                        nc.tensor.matmul(
                            pm1, wsl(ht * 400 + woff, C1), xa,
                            start=(ht == 0), stop=(ht == HT - 1),
                        )
                        nc.tensor.matmul(
                            pm2, wsl(ht * 400 + woff + C1, C2G), xa,
                            start=(ht == 0), stop=(ht == HT - 1),
                        )
                    nc.scalar.activation(
                        out=o1[0:C1, :], in_=pm1, func=gelu,
                        bias=b_sb[0:C1, bc : bc + 1],
                    )
                    nc.scalar.activation(
                        out=o2[0:C2G, :], in_=pm2, func=gelu,
                        bias=b_sb[0:C2G, bc + 2 : bc + 3],
                    )
                    nc.vector.memset(o2[C2G:C2, :], 1.0)

            with (
                tc.tile_pool(name="ps_a", bufs=3, space="PSUM") as ps_a,
                tc.tile_pool(name="ps_s", bufs=4, space="PSUM") as ps_s,
            ):
                # ---- step A: UhT_t[j, x] = sum_i UW[t,i,j] headT[i,x] ----
                for t in range(TAGS):
                    for j0, jw, uh in ((0, C1, uh1), (C1, C2, uh2)):
                        pa = ps_a.tile([jw, NQ], f32, tag="pa")
                        nc.tensor.matmul(
                            pa, usl(t, 0, j0, jw), headT1, start=True, stop=False
                        )
                        nc.tensor.matmul(
                            pa, usl(t, 1, j0, jw), headT2, start=False, stop=True
                        )
                        nc.any.tensor_copy(uh[:, t, :], pa)

                # ---- step B: S_t[x, m] = sum_j UhT_t[j, x] tailT[j, m] ----
                qs = (nc.sync, nc.scalar)
                for qc in range(2):
                    s_sb = (s_sb0, s_sb1)[qc]
                    for t in range(TAGS):
                        pS = ps_s.tile([128, NQ], f32, tag="ps")
                        nc.tensor.matmul(
                            pS, uh1[:, t, qc * 128 : qc * 128 + 128],
                            tailT1[:, qc * 128 : qc * 128 + NQ],
                            start=True, stop=False,
                        )
                        nc.tensor.matmul(
                            pS, uh2[:, t, qc * 128 : qc * 128 + 128],
                            tailT2[:, qc * 128 : qc * 128 + NQ],
                            start=False, stop=True,
                        )
                        nc.any.tensor_copy(s_sb[:, t, :], pS)
                        if t % TS == TS - 1:
                            qs[(qc + t // TS) % 2].dma_start(
                                out=sout[qc, :, t - TS + 1 : t + 1, :],
                                in_=s_sb[:, t - TS + 1 : t + 1, :],
                            )

    nc.compile()
    return nc


def _np_io_dt():
    if io_dt_name() == "bf16":
        import ml_dtypes

        return ml_dtypes.bfloat16
    return np.float32


def _get_nc():
    key = "nc-" + io_dt_name()
    if key not in _cache:
        _cache[key] = _build_nc()
    return _cache[key]


def _install_ntff_hook():
    """Profiling-only (BASSK_TRACE=1): provide antenv.axon_hooks if the
    image lacks it, wired to the libaxon NTFF capture via ctypes."""
    import sys
    import types

    try:
        from antenv.axon_hooks import get_axon_ntff_profile_hook  # noqa: F401

        return
    except ImportError:
        pass
    from trn_agent_boot.trn_boot import _ntff_profile_via_ctypes

    hook = _ntff_profile_via_ctypes("/opt/axon/libaxon_pjrt.so")
    mod = types.ModuleType("antenv.axon_hooks")
    mod._hook = hook
    mod.get_axon_ntff_profile_hook = lambda: mod._hook
    mod.set_axon_ntff_profile_hook = lambda h: setattr(mod, "_hook", h)
    sys.modules["antenv.axon_hooks"] = mod


def _host_prep(state, Wh, bh, Wt, bt, U, Wcat, Wd):
    """Fold U/Wcat/Wd into UW[9,201,201]; pack flat per-core inputs."""
    iodt = _np_io_dt()

    Whp = Wcat[:, :F]
    Wtp = Wcat[:, F:]
    U2 = U.astype(np.float64).copy()
    U2[:, F - 1, :] += Wtp  # head ones-row picks up the tail term
    U2[:, :, F - 1] += Whp  # tail ones-col picks up the head term
    UW = np.einsum("kt,kij->tij", Wd.astype(np.float64), U2).astype(np.float32)

    # uwd[p, (2t+c)*JP + j] = UW[t, ioff_c + p, j]
    uwd = np.zeros((128, 18 * JP), np.float32)
    for t in range(TAGS):
        for c, (ioff, part) in enumerate(((0, C1), (C1, C2))):
            off = (2 * t + c) * JP
            uwd[0:part, off : off + F] = UW[t, ioff : ioff + part, :]
    uwd = np.ascontiguousarray(uwd.astype(iodt))

    # wd[p, ht*400 + {0,200}] = Wh/Wt[ht*128+p, :]
    wd = np.zeros((128, 2400), np.float32)
    for ht in range(HT):
        wd[:, ht * 400 : ht * 400 + 200] = Wh[ht * 128 : (ht + 1) * 128, :]
        wd[:, ht * 400 + 200 : ht * 400 + 400] = Wt[ht * 128 : (ht + 1) * 128, :]
    wd = np.ascontiguousarray(wd.astype(iodt))

    bias4 = np.zeros((128, 4), np.float32)
    bias4[0:C1, 0] = bh[0:C1]
    bias4[0:C1, 1] = bt[0:C1]
    bias4[0:C2G, 2] = bh[C1:BSZ]
    bias4[0:C2G, 3] = bt[C1:BSZ]

    in_maps = []
    for b in range(B):
        for qi in range(N // NQ):
            q0 = qi * NQ
            lo = q0 - W
            xw = np.zeros((NW, HID), np.float32)
            s, e = max(lo, 0), min(q0 + NQ + W, N)
            xw[s - lo : e - lo] = state[b, s:e]
            # xd[p, ht*NW + c] = xw[c, ht*128+p]
            xflat = (
                xw.T.reshape(HT, 128, NW).transpose(1, 0, 2).reshape(128, HT * NW)
            )
            in_maps.append(
                {
                    "xd": np.ascontiguousarray(xflat.astype(iodt)),
                    "wd": wd,
                    "uwd": uwd,
                    "bias4": bias4,
                }
            )
    return in_maps


def _assemble(outs, bd, lengths):
    """outs: NCORES arrays [2, 128, TAGS, NQ] -> scores [B, N, R, TAGS]."""
    scores = np.empty((B, N, R, TAGS), np.float32)
    mi = np.arange(128)[:, None] + np.arange(R)[None, :]  # [128, R]
    for c, S in enumerate(outs):
        b, qi = divmod(c, N // NQ)
        for qc in range(2):
            g = np.take_along_axis(
                S[qc].astype(np.float32), mi[:, None, :], axis=2
            )  # [128, TAGS, R]
            scores[b, qi * NQ + qc * 128 : qi * NQ + (qc + 1) * 128] = g.transpose(
                0, 2, 1
            )
    bdf = bd.astype(np.float32)
    scores += bdf[None, None, None, :]
    # host-side pad mask: masked entries equal bd exactly (0 @ Wd + bd)
    j_idx = np.arange(N)[:, None] + np.arange(R)[None, :] - W  # [N, R]
    in_range = (j_idx >= 0) & (j_idx < N)
    for b in range(B):
        key_ok = in_range & (j_idx < lengths[b])
        q_ok = np.arange(N) < lengths[b]
        pad = ~(key_ok & q_ok[:, None])  # [N, R]
        scores[b][pad] = bdf
    return np.where(np.isfinite(scores), scores, 0.0).astype(np.float32)


def kernel(**inputs):
    state = np.asarray(inputs["state"], np.float32)
    lengths = np.asarray(inputs["lengths"]).astype(np.int64)
    Wh = np.ascontiguousarray(np.asarray(inputs["Wh"], np.float32))
    bh = np.asarray(inputs["bh"], np.float32)
    Wt = np.ascontiguousarray(np.asarray(inputs["Wt"], np.float32))
    bt = np.asarray(inputs["bt"], np.float32)
    U = np.asarray(inputs["U"], np.float32)
    Wcat = np.asarray(inputs["Wcat"], np.float32)
    Wd = np.asarray(inputs["Wd"], np.float32)
    bd = np.asarray(inputs["bd"], np.float32)

    in_maps = _host_prep(state, Wh, bh, Wt, bt, U, Wcat, Wd)
    nc = _get_nc()

    if os.environ.get("BASSK_SIM"):
        from concourse.bass_interp import CoreSim

        outs = []
        for im in in_maps[: int(os.environ.get("BASSK_SIM_N", len(in_maps)))]:
            sim = CoreSim(nc, trace=False)
            for k, v in im.items():
                sim.tensor(k)[:] = v
            sim.simulate()
            outs.append(sim.tensor("sout").copy())
        while len(outs) < NCORES:
            outs.append(outs[-1])
    else:
        trace = bool(os.environ.get("BASSK_TRACE"))
        if trace:
            _install_ntff_hook()
        from concourse.bass_utils import run_bass_kernel_spmd

        try:
            res = run_bass_kernel_spmd(
                nc, in_maps, core_ids=list(range(NCORES)), trace=trace
            )
        except Exception:
            # transient NRT/device hiccups recover on a fresh attempt
            import time

            time.sleep(2.0)
            res = run_bass_kernel_spmd(
                nc, in_maps, core_ids=list(range(NCORES)), trace=trace
            )
        _cache["last_result"] = res
        outs = [r["sout"] for r in res.results]

    return _assemble(outs, bd, lengths)
